# revision 2
# baseline (speedup 1.0000x reference)
"""BinaryLinear Trainium2 kernel, v3: sign-weight-stationary, fp8 sign tiles.

Computes out = x @ (alpha * sign(W)).T + bias where alpha = mean(|W|, axis=1),
for x [4, 2048, 4096] f32, W [4096, 4096] f32, bias [4096] f32.

Sharding: 4-way over tokens x 2-way over out_features = 8 cores. Each core:
x slice [T_c=2048, K=4096], W slice [O_c=2048, K=4096], bias slice [2048];
produces out slice transposed [O_c, T_c] (host permutes back).

Per core: x streams in once (f32), cast on ACT to a resident bf16 tile
[128, KC, T] (k on partitions). W streams per o-tile (128 out rows) as
[128 k, KC, 128 o] f32; ACT computes sign into FP8 stationary tiles (+-1 is
exact in fp8e4, and fp8 LDWEIGHTS streams half the bytes of bf16 with 4x
fast-weight-load packing); DVE abs-reduces |W| over the kc axis, and one
ones-matmul (x 1/K) folds partitions into alpha[o] directly in
o-on-partition layout. Main matmuls: stationary = fp8 sign tile, moving =
resident bf16 x (numerically identical to bf16 x bf16 since signs are
exact). Epilogue: single DVE tensor_scalar per PSUM bank:
out = psum * alpha[p] + bias[p].

Software pipeline: W-chunk DMA + sign + |W|-reduce for o-tile ot+2 are
emitted (and execute) during o-tile ot's matmuls, and the alpha-matmul for
ot+1 sits between the MM blocks of ot and ot+1, so PE/ACT/DVE never stall
at o-tile boundaries.

All arithmetic (cast, sign, alpha, matmul, scale+bias) runs on device; host
marshaling is pure permutation.
"""

import numpy as np

import concourse.bass as bass
import concourse.mybir as mybir
import concourse.tile as tile
from concourse import bacc
from concourse.bass_utils import run_bass_kernel_spmd

F32 = mybir.dt.float32
BF16 = mybir.dt.bfloat16
FP8 = mybir.dt.float8e4

# Full problem shape (hardcoded; kernel.py must be self-contained).
B, S, D_IN, D_OUT = 4, 2048, 4096, 4096
T_FULL = B * S  # 8192 tokens
R_T, C_O = 4, 2  # token-dim shards x out-feature shards = 8 cores
N_CORES = R_T * C_O


def build_nc(K, T, O, reps=1):
    """Per-core Bass program. K=4096 contraction, T=2048 tokens, O=2048 outs."""
    P = 128
    KC = K // P        # 32 k-chunks
    OT = O // P        # 16 o-tiles
    NTT = T // 512     # 4 moving tiles per k-chunk
    XH = 2             # x staged in half-chunks per kc
    assert T % 512 == 0 and K % P == 0 and O % P == 0

    nc = bacc.Bacc("TRN2", target_bir_lowering=False, debug=False)

    xT = nc.dram_tensor("xT", [KC, P, T], F32, kind="ExternalInput")
    wT = nc.dram_tensor("wT", [OT, P, KC, P], F32, kind="ExternalInput")
    biasC = nc.dram_tensor("biasC", [P, OT], F32, kind="ExternalInput")
    out = nc.dram_tensor("out", [O, T], F32, kind="ExternalOutput")

    xT_v = xT.ap()
    wT_v = wT.ap()
    out_v = out.ap().rearrange("(ot p) t -> ot p t", p=P)

    with tile.TileContext(nc) as tc:
        import contextlib

        with contextlib.ExitStack() as ctx:
            const = ctx.enter_context(tc.tile_pool(name="const", bufs=1))
            xbf_pool = ctx.enter_context(tc.tile_pool(name="xbf", bufs=1))
            xstage_pool = ctx.enter_context(tc.tile_pool(name="xstage", bufs=2))
            wstage_pool = ctx.enter_context(tc.tile_pool(name="wstage", bufs=2))
            st_pool = ctx.enter_context(tc.tile_pool(name="st", bufs=3))
            wacc_pool = ctx.enter_context(tc.tile_pool(name="wacc", bufs=2))
            wab_pool = ctx.enter_context(tc.tile_pool(name="wab", bufs=2))
            asb_pool = ctx.enter_context(tc.tile_pool(name="asb", bufs=2))
            out_pool = ctx.enter_context(tc.tile_pool(name="out_sb", bufs=4))
            psum_mm = ctx.enter_context(
                tc.tile_pool(name="psum_mm", bufs=7, space="PSUM")
            )
            psum_al = ctx.enter_context(
                tc.tile_pool(name="psum_al", bufs=1, space="PSUM")
            )

            bias_col = const.tile([P, OT], F32, tag="bias_col")
            ones_k = const.tile([P, 1], BF16, tag="ones_k")
            xbf = xbf_pool.tile([P, KC, T], BF16, tag="xbf")

            def prep_w(ot):
                """W-chunk DMA + sign + |W| kc-reduce for o-tile ot."""
                ws = wstage_pool.tile([P, KC, P], F32, tag="ws", name=f"ws{ot}")
                nc.sync.dma_start(ws[:], wT_v[ot])
                st = st_pool.tile([P, KC, P], FP8, tag="st", name=f"st{ot}")
                nc.scalar.activation(
                    st[:], ws[:], mybir.ActivationFunctionType.Sign
                )
                wacc = wacc_pool.tile([P, P], F32, tag="wacc", name=f"wa{ot}")
                nc.vector.tensor_reduce(
                    wacc[:],
                    ws[:].rearrange("p kc o -> p o kc"),
                    axis=mybir.AxisListType.X,
                    op=mybir.AluOpType.add,
                    apply_absolute_value=True,
                )
                wab = wab_pool.tile([P, P], BF16, tag="wab", name=f"wb{ot}")
                nc.vector.tensor_copy(wab[:], wacc[:])
                return st, wab

            def alpha_mm(wab, ot):
                """Fold partitions of the |W| partial into alpha[o] (x 1/K)."""
                aps = psum_al.tile([P, 1], F32, tag="aps", name=f"ap{ot}")
                nc.tensor.matmul(aps[:], wab[:], ones_k[:], start=True, stop=True)
                asb = asb_pool.tile([P, 1], F32, tag="asb", name=f"as{ot}")
                nc.vector.tensor_copy(asb[:], aps[:])
                return asb

            def body(_=None):
                nc.sync.dma_start(bias_col[:], biasC.ap())
                nc.vector.memset(ones_k[:], 1.0 / K)

                prep = {}
                prep[0] = prep_w(0)
                prep[1] = prep_w(1)

                # ---- x load + cast to resident bf16 (k on partitions)
                for kc in range(KC):
                    for h in range(XH):
                        hs = slice(h * (T // XH), (h + 1) * (T // XH))
                        xs = xstage_pool.tile(
                            [P, T // XH], F32, tag="xs", name=f"xs{kc}_{h}"
                        )
                        nc.sync.dma_start(xs[:], xT_v[kc, :, hs])
                        nc.scalar.copy(xbf[:, kc, hs], xs[:])

                alpha = {0: alpha_mm(prep[0][1], 0)}

                # ---- o-tile loop (prefetch depth 2)
                for ot in range(OT):
                    if ot + 2 < OT:
                        prep[ot + 2] = prep_w(ot + 2)
                    st = prep.pop(ot)[0]
                    ps = [
                        psum_mm.tile([P, 512], F32, tag="ps", name=f"ps{ot}_{tt}")
                        for tt in range(NTT)
                    ]
                    for kc in range(KC):
                        for tt in range(NTT):
                            nc.tensor.matmul(
                                ps[tt][:],
                                st[:, kc, :],
                                xbf[:, kc, tt * 512 : (tt + 1) * 512],
                                start=(kc == 0),
                                stop=(kc == KC - 1),
                            )
                    if ot + 1 < OT:
                        alpha[ot + 1] = alpha_mm(prep[ot + 1][1], ot + 1)
                    asb = alpha.pop(ot)
                    # epilogue: out = psum * alpha[p] + bias[p] (one DVE op/bank)
                    for tt in range(NTT):
                        osb = out_pool.tile(
                            [P, 512], F32, tag="osb", name=f"ob{ot}_{tt}"
                        )
                        nc.vector.tensor_scalar(
                            osb[:],
                            ps[tt][:],
                            asb[:],
                            bias_col[:, ot : ot + 1],
                            op0=mybir.AluOpType.mult,
                            op1=mybir.AluOpType.add,
                        )
                        nc.sync.dma_start(
                            out_v[ot, :, tt * 512 : (tt + 1) * 512], osb[:]
                        )

            if reps == 1:
                body()
            else:
                with tc.For_i(0, reps, 1) as _i:
                    body()

    nc.compile()
    return nc


_NC_CACHE = {}


def _get_nc(key):
    if key not in _NC_CACHE:
        _NC_CACHE[key] = build_nc(*key)
    return _NC_CACHE[key]


def pretile_x(x_slice):
    """[T, K] f32 -> [KC, 128, T] (pure permutation)."""
    T, K = x_slice.shape
    return np.ascontiguousarray(
        x_slice.reshape(T, K // 128, 128).transpose(1, 2, 0)
    )


def pretile_w(w_slice):
    """[O, K] f32 -> [OT, 128, KC, 128] (pure permutation)."""
    O, K = w_slice.shape
    return np.ascontiguousarray(
        w_slice.reshape(O // 128, 128, K // 128, 128).transpose(0, 3, 2, 1)
    )


def make_in_maps(x2, w, b):
    T_c = T_FULL // R_T
    O_c = D_OUT // C_O
    xT_shards = [pretile_x(x2[i * T_c : (i + 1) * T_c, :]) for i in range(R_T)]
    wT_shards = [pretile_w(w[j * O_c : (j + 1) * O_c, :]) for j in range(C_O)]
    bC_shards = [
        np.ascontiguousarray(b[j * O_c : (j + 1) * O_c].reshape(-1, 128).T)
        for j in range(C_O)
    ]
    in_maps = []
    for core in range(N_CORES):
        i, j = core // C_O, core % C_O
        in_maps.append(
            {"xT": xT_shards[i], "wT": wT_shards[j], "biasC": bC_shards[j]}
        )
    return in_maps


def kernel(x, weight_real, bias):
    assert x.shape == (B, S, D_IN) and weight_real.shape == (D_OUT, D_IN)
    x2 = np.ascontiguousarray(
        np.asarray(x, dtype=np.float32).reshape(T_FULL, D_IN)
    )
    w = np.asarray(weight_real, dtype=np.float32)
    b = np.asarray(bias, dtype=np.float32)

    T_c = T_FULL // R_T  # 2048
    O_c = D_OUT // C_O   # 2048

    in_maps = make_in_maps(x2, w, b)
    nc = _get_nc((D_IN, T_c, O_c))
    res = run_bass_kernel_spmd(nc, in_maps, core_ids=list(range(N_CORES)))

    out_full = np.empty((T_FULL, D_OUT), dtype=np.float32)
    for core in range(N_CORES):
        i, j = core // C_O, core % C_O
        out_full[i * T_c : (i + 1) * T_c, j * O_c : (j + 1) * O_c] = res.results[
            core
        ]["out"].T
    return out_full.reshape(B, S, D_OUT)


# revision 3
# speedup vs baseline: 131.8027x; 131.8027x over previous
"""BinaryLinear Trainium2 kernel, v4: v3 + token-half x-stream overlap.

Same math/sharding/marshaling as v3 (sign-weight-stationary, fp8 sign tiles,
alpha via ones-matmul, DVE scale+bias epilogue). The difference is the x
load phase: x streams token-half 0 (all kc) first, then token-half 1, and
the matmul schedule opens (o-tile, token-tile) accumulation groups that only
need the streamed half — so PSUM groups close and recycle mid-stream and the
PE stays busy through the load phase instead of idling behind 8 open banks.

Phase A1 (x half 0 streaming): o-tiles 0-2 x token-tiles {0,1} chase the
stream (6 banks + alpha). Phase A2 (half 1): o-tiles 0,1 x {2,3} chase while
o-tile 3 x {0,1} runs at full speed from the now-resident half 0. Remainder
runs the v3 pipelined o-tile loop.
"""

import numpy as np

import concourse.bass as bass
import concourse.mybir as mybir
import concourse.tile as tile
from concourse import bacc
from concourse.bass_utils import run_bass_kernel_spmd

F32 = mybir.dt.float32
BF16 = mybir.dt.bfloat16
FP8 = mybir.dt.float8e4

B, S, D_IN, D_OUT = 4, 2048, 4096, 4096
T_FULL = B * S
R_T, C_O = 4, 2
N_CORES = R_T * C_O


def build_nc(K, T, O, reps=1):
    P = 128
    KC = K // P        # 32
    OT = O // P        # 16
    NTT = T // 512     # 4
    XH = 2
    assert NTT == 4 and KC >= 2 and OT >= 6

    nc = bacc.Bacc("TRN2", target_bir_lowering=False, debug=False)

    xT = nc.dram_tensor("xT", [KC, P, T], F32, kind="ExternalInput")
    wT = nc.dram_tensor("wT", [OT, P, KC, P], F32, kind="ExternalInput")
    biasC = nc.dram_tensor("biasC", [P, OT], F32, kind="ExternalInput")
    out = nc.dram_tensor("out", [O, T], F32, kind="ExternalOutput")

    xT_v = xT.ap()
    wT_v = wT.ap()
    out_v = out.ap().rearrange("(ot p) t -> ot p t", p=P)

    with tile.TileContext(nc) as tc:
        import contextlib

        with contextlib.ExitStack() as ctx:
            const = ctx.enter_context(tc.tile_pool(name="const", bufs=1))
            xbf_pool = ctx.enter_context(tc.tile_pool(name="xbf", bufs=1))
            xstage_pool = ctx.enter_context(tc.tile_pool(name="xstage", bufs=2))
            wstage_pool = ctx.enter_context(tc.tile_pool(name="wstage", bufs=2))
            st_pool = ctx.enter_context(tc.tile_pool(name="st", bufs=5))
            wacc_pool = ctx.enter_context(tc.tile_pool(name="wacc", bufs=2))
            wab_pool = ctx.enter_context(tc.tile_pool(name="wab", bufs=2))
            asb_pool = ctx.enter_context(tc.tile_pool(name="asb", bufs=5))
            out_pool = ctx.enter_context(tc.tile_pool(name="out_sb", bufs=4))
            psum_mm = ctx.enter_context(
                tc.tile_pool(name="psum_mm", bufs=7, space="PSUM")
            )
            psum_al = ctx.enter_context(
                tc.tile_pool(name="psum_al", bufs=1, space="PSUM")
            )

            bias_col = const.tile([P, OT], F32, tag="bias_col")
            ones_k = const.tile([P, 1], BF16, tag="ones_k")
            xbf = xbf_pool.tile([P, KC, T], BF16, tag="xbf")

            def prep_w(ot):
                ws = wstage_pool.tile([P, KC, P], F32, tag="ws", name=f"ws{ot}")
                nc.sync.dma_start(ws[:], wT_v[ot])
                st = st_pool.tile([P, KC, P], FP8, tag="st", name=f"st{ot}")
                nc.scalar.activation(
                    st[:], ws[:], mybir.ActivationFunctionType.Sign
                )
                wacc = wacc_pool.tile([P, P], F32, tag="wacc", name=f"wa{ot}")
                nc.vector.tensor_reduce(
                    wacc[:],
                    ws[:].rearrange("p kc o -> p o kc"),
                    axis=mybir.AxisListType.X,
                    op=mybir.AluOpType.add,
                    apply_absolute_value=True,
                )
                wab = wab_pool.tile([P, P], BF16, tag="wab", name=f"wb{ot}")
                nc.vector.tensor_copy(wab[:], wacc[:])
                return st, wab

            def alpha_mm(wab, ot):
                aps = psum_al.tile([P, 1], F32, tag="aps", name=f"ap{ot}")
                nc.tensor.matmul(aps[:], wab[:], ones_k[:], start=True, stop=True)
                asb = asb_pool.tile([P, 1], F32, tag="asb", name=f"as{ot}")
                nc.vector.tensor_copy(asb[:], aps[:])
                return asb

            def body(_=None):
                nc.sync.dma_start(bias_col[:], biasC.ap())
                nc.vector.memset(ones_k[:], 1.0 / K)

                st = {}
                wab = {}
                asb = {}
                for j in range(3):
                    st[j], wab[j] = prep_w(j)

                # x stream: token-half-major so half-0 groups close mid-stream
                for h in range(XH):
                    for kc in range(KC):
                        hs = slice(h * (T // XH), (h + 1) * (T // XH))
                        xs = xstage_pool.tile(
                            [P, T // XH], F32, tag="xs", name=f"xs{kc}_{h}"
                        )
                        nc.sync.dma_start(xs[:], xT_v[kc, :, hs])
                        nc.scalar.copy(xbf[:, kc, hs], xs[:])

                for j in range(3):
                    asb[j] = alpha_mm(wab[j], j)

                ps = {}  # (ot, tt) -> psum tile

                def mm(ot, tt, kc):
                    key = (ot, tt)
                    if kc == 0:
                        ps[key] = psum_mm.tile(
                            [P, 512], F32, tag="ps", name=f"ps{ot}_{tt}"
                        )
                    nc.tensor.matmul(
                        ps[key][:],
                        st[ot][:, kc, :],
                        xbf[:, kc, tt * 512 : (tt + 1) * 512],
                        start=(kc == 0),
                        stop=(kc == KC - 1),
                    )

                def epilogue(ot, tt):
                    pt = ps.pop((ot, tt))
                    osb = out_pool.tile(
                        [P, 512], F32, tag="osb", name=f"ob{ot}_{tt}"
                    )
                    nc.vector.tensor_scalar(
                        osb[:],
                        pt[:],
                        asb[ot][:],
                        bias_col[:, ot : ot + 1],
                        op0=mybir.AluOpType.mult,
                        op1=mybir.AluOpType.add,
                    )
                    nc.sync.dma_start(
                        out_v[ot, :, tt * 512 : (tt + 1) * 512], osb[:]
                    )

                # ---- Phase A1: chase x half 0 with (ot 0-2) x (tt 0,1)
                A1 = [(0, 0), (0, 1), (1, 0), (1, 1), (2, 0), (2, 1)]
                for kc in range(KC):
                    for ot, tt in A1:
                        mm(ot, tt, kc)
                st[3], wab[3] = prep_w(3)
                for ot, tt in A1:
                    epilogue(ot, tt)
                asb[3] = alpha_mm(wab[3], 3)

                # ---- Phase A2: chase half 1 with (ot 0,1) x (tt 2,3) while
                # (ot 3) x (tt 0,1) runs full-speed off resident half 0
                A2_chase = [(0, 2), (0, 3), (1, 2), (1, 3)]
                full_units = [(3, tt, kc) for kc in range(KC) for tt in (0, 1)]
                cursor = 0
                for kc in range(KC):
                    for ot, tt in A2_chase:
                        mm(ot, tt, kc)
                    for _ in range(2):
                        if cursor < len(full_units):
                            fot, ftt, fkc = full_units[cursor]
                            mm(fot, ftt, fkc)
                            cursor += 1
                st[4], wab[4] = prep_w(4)
                for ot, tt in A2_chase:
                    epilogue(ot, tt)
                for tt in (0, 1):
                    epilogue(3, tt)
                asb[4] = alpha_mm(wab[4], 4)

                # ---- Remainder: (2,[2,3]), (3,[2,3]) then full o-tiles 4..
                rest = [(2, (2, 3)), (3, (2, 3))] + [
                    (ot, (0, 1, 2, 3)) for ot in range(4, OT)
                ]
                for i, (ot, tts) in enumerate(rest):
                    for kc in range(KC):
                        for tt in tts:
                            mm(ot, tt, kc)
                    # keep the W/sign/alpha pipeline ~2 jobs ahead
                    for pot, _ in rest[i + 1 : i + 3]:
                        if pot not in st:
                            st[pot], wab[pot] = prep_w(pot)
                    for pot, _ in rest[i + 1 : i + 2]:
                        if pot not in asb:
                            asb[pot] = alpha_mm(wab[pot], pot)
                    for tt in tts:
                        epilogue(ot, tt)

            if reps == 1:
                body()
            else:
                with tc.For_i(0, reps, 1) as _i:
                    body()

    nc.compile()
    return nc


_NC_CACHE = {}


def _get_nc(key):
    if key not in _NC_CACHE:
        _NC_CACHE[key] = build_nc(*key)
    return _NC_CACHE[key]


def pretile_x(x_slice):
    T, K = x_slice.shape
    return np.ascontiguousarray(
        x_slice.reshape(T, K // 128, 128).transpose(1, 2, 0)
    )


def pretile_w(w_slice):
    O, K = w_slice.shape
    return np.ascontiguousarray(
        w_slice.reshape(O // 128, 128, K // 128, 128).transpose(0, 3, 2, 1)
    )


def make_in_maps(x2, w, b):
    T_c = T_FULL // R_T
    O_c = D_OUT // C_O
    xT_shards = [pretile_x(x2[i * T_c : (i + 1) * T_c, :]) for i in range(R_T)]
    wT_shards = [pretile_w(w[j * O_c : (j + 1) * O_c, :]) for j in range(C_O)]
    bC_shards = [
        np.ascontiguousarray(b[j * O_c : (j + 1) * O_c].reshape(-1, 128).T)
        for j in range(C_O)
    ]
    in_maps = []
    for core in range(N_CORES):
        i, j = core // C_O, core % C_O
        in_maps.append(
            {"xT": xT_shards[i], "wT": wT_shards[j], "biasC": bC_shards[j]}
        )
    return in_maps


def kernel(x, weight_real, bias):
    assert x.shape == (B, S, D_IN) and weight_real.shape == (D_OUT, D_IN)
    x2 = np.ascontiguousarray(
        np.asarray(x, dtype=np.float32).reshape(T_FULL, D_IN)
    )
    w = np.asarray(weight_real, dtype=np.float32)
    b = np.asarray(bias, dtype=np.float32)

    T_c = T_FULL // R_T
    O_c = D_OUT // C_O

    in_maps = make_in_maps(x2, w, b)
    nc = _get_nc((D_IN, T_c, O_c))
    res = run_bass_kernel_spmd(nc, in_maps, core_ids=list(range(N_CORES)))

    out_full = np.empty((T_FULL, D_OUT), dtype=np.float32)
    for core in range(N_CORES):
        i, j = core // C_O, core % C_O
        out_full[i * T_c : (i + 1) * T_c, j * O_c : (j + 1) * O_c] = res.results[
            core
        ]["out"].T
    return out_full.reshape(B, S, D_OUT)


# revision 4
# speedup vs baseline: 137.5092x; 1.0433x over previous
"""BinaryLinear Trainium2 kernel, v6: 8-way token sharding, W replicated.

Same math/pipeline as v4/v5 (sign-weight-stationary fp8 tiles, ones-matmul
alpha, DVE scale+bias epilogue, token-half x streaming), but sharded 8-way
over tokens with W replicated: per core x [1024, 4096] (16.8MB, half the
load-phase bytes of the 4x2 layout), W full [4096, 4096] streamed per
o-tile, out [4096, 1024] transposed. With T_c=1024 each o-tile has 2 token
tiles, so PSUM banks go ~2x further during the load phase.
"""

import numpy as np

import concourse.mybir as mybir
import concourse.tile as tile
from concourse import bacc
from concourse.bass_utils import run_bass_kernel_spmd

F32 = mybir.dt.float32
BF16 = mybir.dt.bfloat16
FP8 = mybir.dt.float8e4

B, S, D_IN, D_OUT = 4, 2048, 4096, 4096
T_FULL = B * S
R_T, C_O = 8, 1
N_CORES = R_T * C_O


def build_nc(K, T, O, reps=1):
    P = 128
    KC = K // P        # 32
    OT = O // P        # 32
    NTT = T // 512     # 2
    assert NTT == 2 and OT >= 10

    nc = bacc.Bacc("TRN2", target_bir_lowering=False, debug=False)

    xT = nc.dram_tensor("xT", [KC, P, T], F32, kind="ExternalInput")
    wT = nc.dram_tensor("wT", [OT, P, KC, P], F32, kind="ExternalInput")
    biasC = nc.dram_tensor("biasC", [P, OT], F32, kind="ExternalInput")
    out = nc.dram_tensor("out", [O, T], F32, kind="ExternalOutput")

    xT_v = xT.ap()
    wT_v = wT.ap()
    out_v = out.ap().rearrange("(ot p) t -> ot p t", p=P)

    with tile.TileContext(nc) as tc:
        import contextlib

        with contextlib.ExitStack() as ctx:
            const = ctx.enter_context(tc.tile_pool(name="const", bufs=1))
            xbf_pool = ctx.enter_context(tc.tile_pool(name="xbf", bufs=1))
            xstage_pool = ctx.enter_context(tc.tile_pool(name="xstage", bufs=2))
            wstage_pool = ctx.enter_context(tc.tile_pool(name="wstage", bufs=2))
            st_pool = ctx.enter_context(tc.tile_pool(name="st", bufs=9))
            wacc_pool = ctx.enter_context(tc.tile_pool(name="wacc", bufs=2))
            wab_pool = ctx.enter_context(tc.tile_pool(name="wab", bufs=2))
            asb_pool = ctx.enter_context(tc.tile_pool(name="asb", bufs=10))
            out_pool = ctx.enter_context(tc.tile_pool(name="out_sb", bufs=4))
            psum_mm = ctx.enter_context(
                tc.tile_pool(name="psum_mm", bufs=7, space="PSUM")
            )
            psum_al = ctx.enter_context(
                tc.tile_pool(name="psum_al", bufs=1, space="PSUM")
            )

            bias_col = const.tile([P, OT], F32, tag="bias_col")
            ones_k = const.tile([P, 1], BF16, tag="ones_k")
            xbf = xbf_pool.tile([P, KC, T], BF16, tag="xbf")

            def prep_dma(ot):
                ws = wstage_pool.tile([P, KC, P], F32, tag="ws", name=f"ws{ot}")
                nc.sync.dma_start(ws[:], wT_v[ot])
                return ws

            def prep_sign(ot, ws):
                st = st_pool.tile([P, KC, P], FP8, tag="st", name=f"st{ot}")
                nc.scalar.activation(
                    st[:], ws[:], mybir.ActivationFunctionType.Sign
                )
                wacc = wacc_pool.tile([P, P], F32, tag="wacc", name=f"wa{ot}")
                nc.vector.tensor_reduce(
                    wacc[:],
                    ws[:].rearrange("p kc o -> p o kc"),
                    axis=mybir.AxisListType.X,
                    op=mybir.AluOpType.add,
                    apply_absolute_value=True,
                )
                wab = wab_pool.tile([P, P], BF16, tag="wab", name=f"wb{ot}")
                nc.vector.tensor_copy(wab[:], wacc[:])
                return st, wab

            def prep_w(ot):
                return prep_sign(ot, prep_dma(ot))

            def alpha_mm(wab, ot):
                aps = psum_al.tile([P, 1], F32, tag="aps", name=f"ap{ot}")
                nc.tensor.matmul(aps[:], wab[:], ones_k[:], start=True, stop=True)
                asb = asb_pool.tile([P, 1], F32, tag="asb", name=f"as{ot}")
                nc.vector.tensor_copy(asb[:], aps[:])
                return asb

            def body(_=None):
                nc.sync.dma_start(bias_col[:], biasC.ap())
                nc.vector.memset(ones_k[:], 1.0 / K)

                st = {}
                wab = {}
                asb = {}
                for j in range(6):
                    st[j], wab[j] = prep_w(j)
                    asb[j] = alpha_mm(wab[j], j)

                # x stream, token-half-major. W DMAs for the A2 full-speed
                # o-tiles (6-8) interleave with half 0; their sign/|W| ops sit
                # between the two cast blocks on ACT/DVE so they execute at
                # the A1->A2 boundary instead of queueing behind all casts.
                def x_half(h):
                    for kc in range(KC):
                        hs = slice(h * (T // 2), (h + 1) * (T // 2))
                        xs = xstage_pool.tile(
                            [P, T // 2], F32, tag="xs", name=f"xs{kc}_{h}"
                        )
                        nc.sync.dma_start(xs[:], xT_v[kc, :, hs])
                        nc.scalar.copy(xbf[:, kc, hs], xs[:])
                        if h == 0 and kc in (8, 16, 24):
                            wsd[6 + (kc - 8) // 8] = prep_dma(6 + (kc - 8) // 8)

                wsd = {}
                x_half(0)
                for j in (6, 7, 8):
                    st[j], wab[j] = prep_sign(j, wsd.pop(j))
                x_half(1)

                ps = {}

                def mm(ot, tt, kc):
                    key = (ot, tt)
                    if kc == 0:
                        ps[key] = psum_mm.tile(
                            [P, 512], F32, tag="ps", name=f"ps{ot}_{tt}"
                        )
                    nc.tensor.matmul(
                        ps[key][:],
                        st[ot][:, kc, :],
                        xbf[:, kc, tt * 512 : (tt + 1) * 512],
                        start=(kc == 0),
                        stop=(kc == KC - 1),
                    )

                def epilogue(ot, tt):
                    pt = ps.pop((ot, tt))
                    osb = out_pool.tile(
                        [P, 512], F32, tag="osb", name=f"ob{ot}_{tt}"
                    )
                    nc.vector.tensor_scalar(
                        osb[:],
                        pt[:],
                        asb[ot][:],
                        bias_col[:, ot : ot + 1],
                        op0=mybir.AluOpType.mult,
                        op1=mybir.AluOpType.add,
                    )
                    nc.sync.dma_start(
                        out_v[ot, :, tt * 512 : (tt + 1) * 512], osb[:]
                    )

                # ---- Phase A1: chase x half 0 with (ot 0-5, tt 0)
                A1 = [(ot, 0) for ot in range(6)]
                for kc in range(KC):
                    for ot, tt in A1:
                        mm(ot, tt, kc)
                for j in (6, 7, 8):
                    asb[j] = alpha_mm(wab[j], j)
                for ot, tt in A1:
                    epilogue(ot, tt)

                # ---- Phase A2: chase half 1 with (ot 0-3, tt 1) while
                # (ot 6-8, tt 0) run full-speed off resident half 0
                A2_chase = [(ot, 1) for ot in range(4)]
                full_units = [
                    (fot, 0, kc) for fot in (6, 7, 8) for kc in range(KC)
                ]
                cursor = 0
                for kc in range(KC):
                    for ot, tt in A2_chase:
                        mm(ot, tt, kc)
                    for _ in range(3):
                        if cursor < len(full_units):
                            fot, ftt, fkc = full_units[cursor]
                            mm(fot, ftt, fkc)
                            cursor += 1
                            if fkc == KC - 1:
                                epilogue(fot, ftt)
                for ot, tt in A2_chase:
                    epilogue(ot, tt)

                # ---- Remainder
                rest = [(ot, (1,)) for ot in (4, 5, 6, 7, 8)] + [
                    (ot, (0, 1)) for ot in range(9, OT)
                ]
                for i, (ot, tts) in enumerate(rest):
                    for kc in range(KC):
                        for tt in tts:
                            mm(ot, tt, kc)
                    for pot, _ in rest[i + 1 : i + 3]:
                        if pot not in st:
                            st[pot], wab[pot] = prep_w(pot)
                    for pot, _ in rest[i + 1 : i + 2]:
                        if pot not in asb:
                            asb[pot] = alpha_mm(wab[pot], pot)
                    for tt in tts:
                        epilogue(ot, tt)

            if reps == 1:
                body()
            else:
                with tc.For_i(0, reps, 1) as _i:
                    body()

    nc.compile()
    return nc


_NC_CACHE = {}


def _get_nc(key):
    if key not in _NC_CACHE:
        _NC_CACHE[key] = build_nc(*key)
    return _NC_CACHE[key]


def pretile_x(x_slice):
    T, K = x_slice.shape
    return np.ascontiguousarray(
        x_slice.reshape(T, K // 128, 128).transpose(1, 2, 0)
    )


def pretile_w(w_slice):
    O, K = w_slice.shape
    return np.ascontiguousarray(
        w_slice.reshape(O // 128, 128, K // 128, 128).transpose(0, 3, 2, 1)
    )


def make_in_maps(x2, w, b):
    T_c = T_FULL // R_T
    xT_shards = [pretile_x(x2[i * T_c : (i + 1) * T_c, :]) for i in range(R_T)]
    wT_full = pretile_w(w)
    bC = np.ascontiguousarray(b.reshape(-1, 128).T)
    return [
        {"xT": xT_shards[i], "wT": wT_full, "biasC": bC} for i in range(N_CORES)
    ]


def kernel(x, weight_real, bias):
    assert x.shape == (B, S, D_IN) and weight_real.shape == (D_OUT, D_IN)
    x2 = np.ascontiguousarray(
        np.asarray(x, dtype=np.float32).reshape(T_FULL, D_IN)
    )
    w = np.asarray(weight_real, dtype=np.float32)
    b = np.asarray(bias, dtype=np.float32)

    T_c = T_FULL // R_T  # 1024

    in_maps = make_in_maps(x2, w, b)
    nc = _get_nc((D_IN, T_c, D_OUT))
    res = run_bass_kernel_spmd(nc, in_maps, core_ids=list(range(N_CORES)))

    out_full = np.empty((T_FULL, D_OUT), dtype=np.float32)
    for i in range(N_CORES):
        out_full[i * T_c : (i + 1) * T_c, :] = res.results[i]["out"].T
    return out_full.reshape(B, S, D_OUT)
